# revision 30
# baseline (speedup 1.0000x reference)
"""MixedQLinear (QUIK-style int4+fp16 outlier linear) on 8 TRN2 NeuronCores.

Sharding: token-parallel. x [4,2048,4096] -> 8192 tokens, 1024 per core;
weights replicated. Each core quantizes its tokens, runs the int4 GEMM in
fp8e4 DoubleRow mode (exact: products of ints in [-8,7] are exact in the
e6m3 PE datapath, accumulated in fp32 PSUM), and writes its [1024,4096]
output slice. Host concatenates.

Key algebra: with r = clip(round((x-mn)/scale),0,15) - 8,
  out = [ sum_k r*Wint + (fp_x/scale) @ (Wfp/ws)^T + (8+mn/scale)*(rw/ws)
          + (1/scale)*(bias/ws) ] * scale * ws
so the zero-point correction and bias ride as extra contraction rows of the
fp-outlier matmul (scaled by 1/scale per token), and dequant is one scaled
PSUM copy plus one multiply by ws.

Schedule notes (from trace analysis):
- Each DMA instruction drains one queue at ~25 GB/s; the HWDGE rings execute
  their DMA triggers in FIFO order. The sync ring therefore carries ONLY
  dependency-free loads (split into sub-512KB pieces, priority-ordered);
  dependent transfers ride the scalar ring.
- Per-token min/max comes from the transposed x tiles (the same tiles the
  quantizer reads): elementwise min/max trees split across Vector and
  GpSimd, a PE-transpose of the [128,512] accumulators into PSUM, then
  per-token-tile free-axis reduces. Broadcast rows for the quantizer are
  produced by ones[1,128] (x) row[1,512] matmuls into spare PSUM rotations.
- The int GEMM interleaves 4 psum banks per rt-chunk (stationary shared);
  measured matmul issue rate is ~216-222 ns per 512-col stream for both f16
  and DoubleRow. Phase M is emitted as (g0:t0-3), then half-1 stats, then
  (g0:t4-7), (g1:*) so the tensor stream never waits on half-1 stats.
"""

import numpy as np
import ml_dtypes
import concourse.bass as bass
import concourse.tile as tile
import concourse.mybir as mybir
from concourse.bass_utils import run_bass_kernel_spmd
from bass_rust import ScopedClock, SyncInfo
from concourse.alu_op_type import AluOpType

# ---------------------------------------------------------------------------
# Workaround: this toolchain's walrus accepts at most one sync-wait on a
# TPB_CTRL (Drain) instruction; Tile's tail drain attaches one wait per
# active DMA queue. Split it into a chain of single-wait drains.
def _drain_and_barrier(self, tick_clock, wait_clock):
    drain_inst = self.nc.sync.drain()
    wait_clock.add_sem_waits(
        drain_inst.ins, ScopedClock({None: tick_clock.global_clock})
    )
    si = drain_inst.ins.sync_info
    ow = list(si.on_wait) if si is not None else []
    if len(ow) > 1:
        si.on_wait = [ow[0]]
        for w in ow[1:]:
            d2 = self.nc.sync.drain()
            d2.ins.sync_info = SyncInfo(on_wait=[w], on_update=[])
    self.nc.all_engine_barrier()
    assert self.sems is not None
    popped = self.nc._tile_sem_poison_stack.pop()
    assert popped is self._sem_poison
    self.nc.clear_and_free_semaphores(list(self.sems.allocated().values()))
    self.nc.all_engine_barrier()


tile.TileContext._drain_and_barrier = _drain_and_barrier


def _split_multiwait_instructions(nc):
    """Walrus here allows only one sync-wait per instruction: hoist extra
    waits onto same-engine NOPs inserted immediately before."""
    ctr = 0
    for fn in nc.m.functions:
        for bb in fn.blocks:
            insts = bb.instructions
            out = []
            changed = False
            for ins in insts:
                si = getattr(ins, "sync_info", None)
                ow = list(si.on_wait) if si is not None else []
                if len(ow) > 1:
                    changed = True
                    for w in ow[:-1]:
                        ctr += 1
                        out.append(
                            mybir.InstNoOp(
                                name=f"mwsplit-{ctr}",
                                sync_info=SyncInfo(on_wait=[w], on_update=[]),
                                engine=ins.engine,
                                bass_nofuse=True,
                            )
                        )
                    si.on_wait = [ow[-1]]
                out.append(ins)
            if changed:
                bb.instructions = out
# ---------------------------------------------------------------------------

N_CORES = 8
B, S, IN, OUT, FP = 4, 2048, 4096, 4096, 256
INT = IN - FP                    # 3840 int features (compact order)
NT = (B * S) // N_CORES          # 1024 tokens per core
P = 128
KC = INT // P                    # 30 feature chunks of 128
CC = KC // 2                     # 15 DoubleRow chunks of 256
NOUT = 8                         # out-feature chunks
NSZ = OUT // NOUT                # 512
NGRP = 2                         # n-chunk groups (4 chunks each, 4 psum banks)
GS = NOUT // NGRP                # 4
HT = 2                           # token halves (512) for quantize layout
HSZ = NT // HT                   # 512
TOKT = NT // P                   # 8 token tiles of 128
TH = TOKT // HT                  # 4 token tiles per half

# Measured: the vector engine's f32->i8 output conversion is round-to-
# nearest-even, matching jnp.round exactly; no truncation compensation.
TRUNC_I8 = False

f16 = mybir.dt.float16
f32 = mybir.dt.float32
f8 = mybir.dt.float8e4
i8 = mybir.dt.int8

_prog_cache = {}


def _build_program():
    nc = bass.Bass()
    xst = nc.declare_dram_parameter("xst", [INT, NT], f16, isOutput=False)
    fpxt = nc.declare_dram_parameter("fpxt", [FP, NT], f16, isOutput=False)
    w8n = nc.declare_dram_parameter("w8n", [NOUT, P, CC, 2, NSZ], f8, isOutput=False)
    wfp = nc.declare_dram_parameter("wfp", [FP, OUT], f16, isOutput=False)
    wfp2 = nc.declare_dram_parameter("wfp2", [2, OUT], f16, isOutput=False)
    wsb_d = nc.declare_dram_parameter("wsb", [P, OUT], f16, isOutput=False)
    ident_d = nc.declare_dram_parameter("ident", [P, P], f16, isOutput=False)
    out_d = nc.declare_dram_parameter("out", [NT, OUT], f16, isOutput=True)

    with tile.TileContext(nc) as tc:
        with (
            tc.tile_pool(name="const", bufs=1) as cpool,
            tc.tile_pool(name="xt", bufs=1) as xtpool,
            tc.tile_pool(name="rt", bufs=1) as rtpool,
            tc.tile_pool(name="wp", bufs=1) as wpool,
            tc.tile_pool(name="st", bufs=2) as stpool,
            tc.tile_pool(name="s1", bufs=1) as s1pool,
            tc.tile_pool(name="q", bufs=3) as qpool,
            tc.tile_pool(name="dq", bufs=4) as dqpool,
            tc.tile_pool(name="psA", bufs=2, space="PSUM") as ppoolA,
            tc.tile_pool(name="psB", bufs=1, space="PSUM") as ppoolB,
            tc.tile_pool(name="tr", bufs=1, space="PSUM") as trpool,
            tc.tile_pool(name="dram", bufs=1, space="DRAM") as dpool,
        ):
            # ---- sync ring: all dependency-free loads, priority order ----
            # Loads alternate between the two HWDGE rings (sync/scalar) in
            # need-by order: xt half0 + weight quarter 0 first, remaining
            # weight quarters next, fp/dequant constants, then xt half1.
            rings = [nc.sync, nc.scalar]

            def load_xt(h):
                hs = slice(h * HSZ, (h + 1) * HSZ)
                tiles = []
                for k in range(KC):
                    t_ = xtpool.tile([P, HSZ], f16, name=f"xt{k}", tag=f"xt{k}")
                    rings[k % 2].dma_start(t_[:], xst[k * P : (k + 1) * P, hs])
                    tiles.append(t_)
                return tiles

            QR0 = [(0, 4), (4, 8), (8, 12), (12, CC)]

            def load_wq(g, wq):
                for qi in range(4):
                    c0, c1 = QR0[qi]
                    for s in range(GS):
                        wt = wpool.tile(
                            [P, c1 - c0, 2, NSZ], f8,
                            name=f"w{s}q{qi}", tag=f"w{s}q{qi}",
                        )
                        rings[s % 2].dma_start(wt[:], w8n[g * GS + s, :, c0:c1])
                        wq[s][qi] = wt

            xts = [None, None]
            xts[0] = load_xt(0)
            wqs = [[None] * 4 for _ in range(GS)]
            load_wq(0, wqs)
            fpx0 = cpool.tile([P, NT], f16, tag="fpx0")
            nc.sync.dma_start(fpx0[:], fpxt[0:P, :])
            fpx1 = cpool.tile([P, NT], f16, tag="fpx1")
            nc.scalar.dma_start(fpx1[:], fpxt[P:FP, :])
            ident = cpool.tile([P, P], f16, tag="ident")
            nc.scalar.dma_start(ident[:], ident_d[:])
            wfp0_s = cpool.tile([P, OUT], f16, tag="wfp0")
            nc.sync.dma_start(wfp0_s[:, 0 : OUT // 2], wfp[0:P, 0 : OUT // 2])
            nc.scalar.dma_start(wfp0_s[:, OUT // 2 :], wfp[0:P, OUT // 2 :])
            wfp1_s = cpool.tile([P, OUT], f16, tag="wfp1")
            nc.sync.dma_start(wfp1_s[:, 0 : OUT // 2], wfp[P:FP, 0 : OUT // 2])
            nc.scalar.dma_start(wfp1_s[:, OUT // 2 :], wfp[P:FP, OUT // 2 :])
            wfp2_s = cpool.tile([2, OUT], f16, tag="wfp2")
            nc.sync.dma_start(wfp2_s[:], wfp2[:])
            wsB = cpool.tile([P, OUT], f16, tag="wsB")
            nc.sync.dma_start(wsB[:, 0 : OUT // 2], wsb_d[:, 0 : OUT // 2])
            nc.scalar.dma_start(wsB[:, OUT // 2 :], wsb_d[:, OUT // 2 :])
            # xt half-1 loads are emitted after the half-0 quantize section:
            # their triggers wait on half-0 consumption and would otherwise
            # block the scalar ring's later work.

            fpt2 = cpool.tile([2, NT], f16, tag="fpt2")
            ones_t = cpool.tile([1, P], f32, tag="ones")
            nc.vector.memset(ones_t[:], 1.0)
            ident32 = cpool.tile([P, P], f32, tag="ident32")
            nc.scalar.copy(ident32[:], ident[:])

            rt = [
                [
                    rtpool.tile(
                        [P, 2, HSZ], f8, name=f"rt{c}_{h}", tag=f"rt{c}_{h}"
                    )
                    for h in range(HT)
                ]
                for c in range(CC)
            ]
            scl = [None] * TOKT
            trees = {}
            rows = {}
            trs = {}

            # ---- per-half stats + quantize, split into emission slots ----
            # slot 0: tree part 1; slot 1: tree part 2; slot 2: transposes +
            # reduces + chains + stat stores; slot 3: rows + broadcasts +
            # fp scaling + quantize. Emitting the slots of half 1 between
            # phase-M(g0) token iterations keeps every engine stream
            # drained while half-0 GEMM work proceeds.
            def half_tree(h, part):
                xt = xts[h]
                if part == 0:
                    mna = stpool.tile([P, HSZ], f16, tag="mna")
                    nc.vector.tensor_tensor(
                        mna[:], xt[0][:], xt[1][:], AluOpType.min
                    )
                    mxa = stpool.tile([P, HSZ], f16, tag="mxa")
                    nc.vector.tensor_tensor(
                        mxa[:], xt[0][:], xt[1][:], AluOpType.max
                    )
                    trees[h] = (mna, mxa)
                    rng = range(2, KC // 2)
                else:
                    mna, mxa = trees[h]
                    rng = range(KC // 2, KC)
                for k in rng:
                    nc.vector.tensor_tensor(mna[:], mna[:], xt[k][:], AluOpType.min)
                    nc.vector.tensor_tensor(mxa[:], mxa[:], xt[k][:], AluOpType.max)
            def half_stats(h):
                mna, mxa = trees[h]
                # psum row tiles for transposed mnq/inv stat columns (f32,
                # partition 0); two pi2 rotations, no extra PSUM bank
                rowtA = ppoolA.tile([P, NSZ], f32, name="pi2", tag="pi2")
                rowtB = ppoolA.tile([P, NSZ], f32, name="pi2", tag="pi2")
                rows[h] = (rowtA, rowtB)
                # PE transpose both accumulators into one psum bank
                tr_t = trpool.tile([P, 2 * HSZ], f16, tag="tr")
                trs[h] = tr_t
                for b in range(TH):
                    bs = slice(b * P, (b + 1) * P)
                    nc.tensor.transpose(tr_t[:, bs], mna[:, bs], ident[:])
                for b in range(TH):
                    bs = slice(b * P, (b + 1) * P)
                    nc.tensor.transpose(
                        tr_t[:, HSZ + b * P : HSZ + (b + 1) * P],
                        mxa[:, bs], ident[:],
                    )
                # per-token-tile stat columns + scale chain
                for b in range(TH):
                    t = h * TH + b
                    ts_ = slice(t * P, (t + 1) * P)
                    mn_t = s1pool.tile([P, 1], f32, name=f"mn{t}", tag=f"mn{t}")
                    nc.vector.tensor_reduce(
                        mn_t[:], tr_t[:, b * P : (b + 1) * P],
                        mybir.AxisListType.X, AluOpType.min,
                    )
                    mx_t = s1pool.tile([P, 1], f32, name=f"mx{t}", tag=f"mx{t}")
                    nc.vector.tensor_reduce(
                        mx_t[:], tr_t[:, HSZ + b * P : HSZ + (b + 1) * P],
                        mybir.AxisListType.X, AluOpType.max,
                    )
                    sc_t = s1pool.tile([P, 1], f32, name=f"sc{t}", tag=f"sc{t}")
                    nc.vector.tensor_tensor(
                        sc_t[:], mx_t[:], mn_t[:], AluOpType.subtract
                    )
                    nc.vector.tensor_scalar(
                        sc_t[:], sc_t[:], 1.0 / 15.0, 1e-8,
                        AluOpType.mult, AluOpType.max,
                    )
                    inv_t = s1pool.tile([P, 1], f32, name=f"inv{t}", tag=f"inv{t}")
                    nc.vector.reciprocal(inv_t[:], sc_t[:])
                    nwt = s1pool.tile([P, 1], f32, name=f"nw{t}", tag=f"nw{t}")
                    nc.vector.tensor_tensor(nwt[:], sc_t[:], inv_t[:], AluOpType.mult)
                    nc.vector.tensor_scalar(
                        nwt[:], nwt[:], -1.0, 2.0, AluOpType.mult, AluOpType.add
                    )
                    nc.vector.tensor_tensor(inv_t[:], inv_t[:], nwt[:], AluOpType.mult)
                    mq_t = s1pool.tile([P, 1], f32, name=f"mq{t}", tag=f"mq{t}")
                    if TRUNC_I8:
                        nc.vector.tensor_scalar(
                            mq_t[:], sc_t[:], -0.5, None, AluOpType.mult
                        )
                        nc.vector.tensor_tensor(
                            mq_t[:], mq_t[:], mn_t[:], AluOpType.add
                        )
                    else:
                        nc.vector.tensor_scalar(
                            mq_t[:], mn_t[:], 1.0, None, AluOpType.mult
                        )
                    zr_t = s1pool.tile([P, 1], f32, name=f"zrf{t}", tag=f"zrf{t}")
                    nc.vector.tensor_tensor(zr_t[:], mn_t[:], inv_t[:], AluOpType.mult)
                    zr16 = s1pool.tile([P, 1], f16, name=f"zr{t}", tag=f"zr{t}")
                    nc.vector.tensor_scalar(
                        zr16[:], zr_t[:], 1.0, 8.0, AluOpType.mult, AluOpType.add
                    )
                    iv16 = s1pool.tile([P, 1], f16, name=f"iv{t}", tag=f"iv{t}")
                    nc.vector.tensor_scalar(
                        iv16[:], inv_t[:], 0.0, None, AluOpType.add
                    )
                    scl[t] = sc_t
                    # PE-transpose the stat columns into row form (all at
                    # partition 0): mnq/inv (f32) into the pi2 rotations,
                    # zrow/invrow (f16) over the retired tree-transpose
                    # psum column ranges.
                    bs = slice(b * P, (b + 1) * P)
                    nc.tensor.transpose(rowtA[0:1, bs], mq_t[:], ident32[:])
                    nc.tensor.transpose(rowtB[0:1, bs], inv_t[:], ident32[:])
                    nc.tensor.transpose(tr_t[0:1, bs], zr16[:], ident[:])
                    nc.tensor.transpose(
                        tr_t[0:1, HSZ + b * P : HSZ + (b + 1) * P], iv16[:],
                        ident[:],
                    )

            def half_quant(h):
                hs = slice(h * HSZ, (h + 1) * HSZ)
                xt = xts[h]
                rowtA, rowtB = rows[h]
                tr_t = trs[h]
                # psum rows -> SBUF (partition-0 aligned copies)
                rwa = stpool.tile([1, HSZ], f32, tag="rwa")
                nc.vector.tensor_scalar(
                    rwa[:], rowtA[0:1, :], 0.0, None, AluOpType.add
                )
                rwb = stpool.tile([1, HSZ], f32, tag="rwb")
                nc.vector.tensor_scalar(
                    rwb[:], rowtB[0:1, :], 0.0, None, AluOpType.add
                )
                row16 = stpool.tile([1, 2 * HSZ], f16, tag="row16")
                nc.scalar.copy(row16[:], tr_t[0:1, :])
                nc.scalar.dma_start(fpt2[0:1, hs], row16[0:1, 0:HSZ])
                nc.scalar.dma_start(fpt2[1:2, hs], row16[0:1, HSZ:])
                bc0 = ppoolA.tile([P, NSZ], f32, name="pi0", tag="pi0")
                nc.tensor.matmul(
                    bc0[:], ones_t[:], rwa[:], start=True, stop=True
                )
                mnqB = stpool.tile([P, HSZ], f32, tag="mnqB")
                nc.vector.tensor_scalar(mnqB[:], bc0[:], 0.0, None, AluOpType.add)
                bc1 = ppoolA.tile([P, NSZ], f32, name="pi1", tag="pi1")
                nc.tensor.matmul(
                    bc1[:], ones_t[:], rwb[:], start=True, stop=True
                )
                invB = stpool.tile([P, HSZ], f32, tag="invB")
                nc.vector.tensor_scalar(invB[:], bc1[:], 0.0, None, AluOpType.add)
                # fp outlier rows scaled by inv (in place)
                nc.vector.tensor_tensor(
                    fpx0[:, hs], fpx0[:, hs], invB[:], AluOpType.mult
                )
                nc.vector.tensor_tensor(
                    fpx1[:, hs], fpx1[:, hs], invB[:], AluOpType.mult
                )
                # quantize chunks: sub+mult->i8 on vector, i8->f8 on scalar
                for k in range(KC):
                    q = qpool.tile([P, HSZ], f32, name="q", tag="q")
                    nc.vector.tensor_tensor(
                        q[:], xt[k][:], mnqB[:], AluOpType.subtract
                    )
                    r8i = qpool.tile([P, HSZ], i8, name="r8", tag="r8")
                    nc.vector.tensor_tensor(r8i[:], q[:], invB[:], AluOpType.mult)
                    nc.scalar.activation(
                        rt[k // 2][h][:, k % 2, :], r8i[:],
                        mybir.ActivationFunctionType.Copy, bias=-8.0,
                    )

            # ---- phase M: one token-tile iteration ----------------------
            # weights come as 4 quarter-tiles per n-chunk slot (c-ranges
            # 0:4, 4:8, 8:12, 12:15) so group-1 loads can start as soon as
            # group-0's early c-chunks retire.
            QR = [(0, 4), (4, 8), (8, 12), (12, CC)]

            def mm_c(ps, t, c, s, wq, start):
                h = t // TH
                tsl = slice((t % TH) * P, (t % TH) * P + P)
                qi = min(c // 4, 3)
                nc.tensor.matmul(
                    ps[:], rt[c][h][:, :, tsl], wq[s][qi][:, c - QR[qi][0]],
                    start=start, stop=False,
                    perf_mode=mybir.MatmulPerfMode.DoubleRow,
                )

            def mm_fp(ps, g, t, s):
                ts_ = slice(t * P, (t + 1) * P)
                ns = slice((g * GS + s) * NSZ, (g * GS + s + 1) * NSZ)
                nc.tensor.matmul(
                    ps[:], fpx0[:, ts_], wfp0_s[:, ns], start=False, stop=False
                )
                nc.tensor.matmul(
                    ps[:], fpx1[:, ts_], wfp1_s[:, ns], start=False, stop=False
                )
                nc.tensor.matmul(
                    ps[:], fpt2[:, ts_], wfp2_s[:, ns], start=False, stop=True
                )

            def dequant(ps, g, t, s):
                ts_ = slice(t * P, (t + 1) * P)
                ns = slice((g * GS + s) * NSZ, (g * GS + s + 1) * NSZ)
                td = dqpool.tile([P, NSZ], f16, tag="td")
                nc.scalar.activation(
                    td[:], ps[:], mybir.ActivationFunctionType.Copy,
                    scale=scl[t][:, 0:1],
                )
                outt = dqpool.tile([P, NSZ], f16, tag="outt")
                nc.vector.tensor_tensor(
                    outt[:], td[:], wsB[:, ns], AluOpType.mult
                )
                nc.scalar.dma_start(out_d[ts_, ns], outt[:])

            def phase_m_t(g, t, wq):
                psums = []
                for s in range(GS):
                    pool = ppoolA if s < 3 else ppoolB
                    psums.append(
                        pool.tile([P, NSZ], f32, name=f"pi{s}", tag=f"pi{s}")
                    )
                for c in range(CC):
                    for s in range(GS):
                        mm_c(psums[s], t, c, s, wq, c == 0)
                for s in range(GS):
                    mm_fp(psums[s], g, t, s)
                for s in range(GS):
                    dequant(psums[s], g, t, s)

            def phase_m_pair(g, ta, tb, wq):
                """Interleave two token-tiles' c-loops (7 matmuls per chunk)
                so a quantize-paced rt stream keeps the tensor engine fed.
                tb's s=3 group runs after ta's s=3 psum bank is drained."""
                psa = []
                for s in range(GS):
                    pool = ppoolA if s < 3 else ppoolB
                    psa.append(
                        pool.tile([P, NSZ], f32, name=f"pi{s}", tag=f"pi{s}")
                    )
                psb = [
                    ppoolA.tile([P, NSZ], f32, name=f"pi{s}", tag=f"pi{s}")
                    for s in range(3)
                ]
                for c in range(CC):
                    for s in range(GS):
                        mm_c(psa[s], ta, c, s, wq, c == 0)
                    for s in range(3):
                        mm_c(psb[s], tb, c, s, wq, c == 0)
                for s in range(GS):
                    mm_fp(psa[s], g, ta, s)
                for s in range(3):
                    mm_fp(psb[s], g, tb, s)
                for s in range(GS):
                    dequant(psa[s], g, ta, s)
                for s in range(3):
                    dequant(psb[s], g, tb, s)
                ps3 = ppoolB.tile([P, NSZ], f32, name="pi3", tag="pi3")
                for c in range(CC):
                    mm_c(ps3, tb, c, 3, wq, c == 0)
                mm_fp(ps3, g, tb, 3)
                dequant(ps3, g, tb, 3)

            # ---- schedule ----------------------------------------------
            half_tree(0, 0)
            half_tree(0, 1)
            half_stats(0)
            half_quant(0)
            xts[1] = load_xt(1)
            phase_m_pair(0, 0, 1, wqs)
            half_tree(1, 0)
            phase_m_t(0, 2, wqs)
            half_tree(1, 1)
            phase_m_t(0, 3, wqs)
            half_stats(1)
            half_quant(1)
            phase_m_pair(0, 4, 5, wqs)
            phase_m_t(0, 6, wqs)
            phase_m_t(0, 7, wqs)
            wqs2 = [[None] * 4 for _ in range(GS)]
            load_wq(1, wqs2)
            for t in range(TOKT):
                phase_m_t(1, t, wqs2)
    _split_multiwait_instructions(nc)
    return nc


def _get_program():
    if "nc" not in _prog_cache:
        _prog_cache["nc"] = _build_program()
    return _prog_cache["nc"]


def _prep_shared(int_weight, fp_weight, bias, weights_scales, reduced_w):
    """Host-side weight layouts (shared across cores)."""
    wint = np.asarray(int_weight).astype(np.float32)          # [OUT, INT]
    ws32 = np.asarray(weights_scales, dtype=np.float32).reshape(OUT, 1)
    # w8n[n, p, c, j, o'] = wint[n*NSZ+o', c*256 + j*128 + p]
    wT = np.ascontiguousarray(wint.T)                         # [INT, OUT]
    w8 = wT.reshape(CC, 2, P, NOUT, NSZ).transpose(3, 2, 0, 1, 4)
    w8n = np.ascontiguousarray(w8).astype(ml_dtypes.float8_e4m3)
    # fp weights scaled by 1/ws, transposed
    wfpT = (np.asarray(fp_weight, dtype=np.float32) / ws32).T  # [FP, OUT]
    wfp = np.ascontiguousarray(wfpT).astype(np.float16)
    # extra contraction rows: [rw/ws = sum_k wint (exact ints), bias/ws]
    row_rw = wint.sum(axis=1)                                  # [OUT]
    row_bias = np.asarray(bias, dtype=np.float32) / ws32[:, 0]
    wfp2 = np.stack([row_rw, row_bias]).astype(np.float16)
    wsb = np.ascontiguousarray(
        np.broadcast_to(
            np.asarray(weights_scales, dtype=np.float16).reshape(1, OUT),
            (P, OUT),
        )
    )
    ident = np.eye(P, dtype=np.float16)
    return w8n, wfp, wfp2, wsb, ident


def _make_in_maps(x, int_weight, fp_weight, bias, weights_scales, reduced_w,
                  int_indices, fp_indices):
    x2 = np.asarray(x, dtype=np.float16).reshape(-1, IN)
    ii = np.asarray(int_indices).astype(np.int64)
    fi = np.asarray(fp_indices).astype(np.int64)

    w8n, wfp, wfp2, wsb, ident = _prep_shared(
        int_weight, fp_weight, bias, weights_scales, reduced_w
    )
    xint = x2[:, ii]                                           # [N, INT]
    xfp = x2[:, fi]                                            # [N, FP]

    in_maps = []
    for c in range(N_CORES):
        sl = slice(c * NT, (c + 1) * NT)
        in_maps.append({
            "xst": np.ascontiguousarray(xint[sl].T),
            "fpxt": np.ascontiguousarray(xfp[sl].T),
            "w8n": w8n,
            "wfp": wfp,
            "wfp2": wfp2,
            "wsb": wsb,
            "ident": ident,
        })
    return in_maps


def kernel(x, int_weight, fp_weight, bias, weights_scales, reduced_w,
           int_indices, fp_indices):
    in_maps = _make_in_maps(
        x, int_weight, fp_weight, bias, weights_scales, reduced_w,
        int_indices, fp_indices,
    )
    nc = _get_program()
    res = run_bass_kernel_spmd(nc, in_maps, list(range(N_CORES)))
    out = np.concatenate(
        [res.results[c]["out"] for c in range(N_CORES)], axis=0
    )
    return out.reshape(B, S, OUT).astype(np.float16)


# revision 36
# speedup vs baseline: 1.0329x; 1.0329x over previous
"""MixedQLinear (QUIK-style int4+fp16 outlier linear) on 8 TRN2 NeuronCores.

Sharding: token-parallel. x [4,2048,4096] -> 8192 tokens, 1024 per core;
weights replicated. Each core quantizes its tokens, runs the int4 GEMM in
fp8e4 DoubleRow mode (exact: products of ints in [-8,7] are exact in the
e6m3 PE datapath, accumulated in fp32 PSUM), and writes its [1024,4096]
output slice. Host concatenates.

Key algebra: with r = clip(round((x-mn)/scale),0,15) - 8,
  out = [ sum_k r*Wint + (fp_x/scale) @ (Wfp/ws)^T + (8+mn/scale)*(rw/ws)
          + (1/scale)*(bias/ws) ] * scale * ws
so the zero-point correction and bias ride as extra contraction rows of the
fp-outlier matmul (scaled by 1/scale per token), and dequant is one scaled
PSUM copy plus one multiply by ws.

Schedule notes (from trace analysis):
- Each DMA instruction drains one queue at ~25 GB/s; the HWDGE rings execute
  their DMA triggers in FIFO order. The sync ring therefore carries ONLY
  dependency-free loads (split into sub-512KB pieces, priority-ordered);
  dependent transfers ride the scalar ring.
- Per-token min/max comes from the transposed x tiles (the same tiles the
  quantizer reads): elementwise min/max trees split across Vector and
  GpSimd, a PE-transpose of the [128,512] accumulators into PSUM, then
  per-token-tile free-axis reduces. Broadcast rows for the quantizer are
  produced by ones[1,128] (x) row[1,512] matmuls into spare PSUM rotations.
- The int GEMM interleaves 4 psum banks per rt-chunk (stationary shared);
  measured matmul issue rate is ~216-222 ns per 512-col stream for both f16
  and DoubleRow. Phase M is emitted as (g0:t0-3), then half-1 stats, then
  (g0:t4-7), (g1:*) so the tensor stream never waits on half-1 stats.
"""

import numpy as np
import ml_dtypes
import concourse.bass as bass
import concourse.tile as tile
import concourse.mybir as mybir
from concourse.bass_utils import run_bass_kernel_spmd
from bass_rust import ScopedClock, SyncInfo
from concourse.alu_op_type import AluOpType

# ---------------------------------------------------------------------------
# Workaround: this toolchain's walrus accepts at most one sync-wait on a
# TPB_CTRL (Drain) instruction; Tile's tail drain attaches one wait per
# active DMA queue. Split it into a chain of single-wait drains.
def _drain_and_barrier(self, tick_clock, wait_clock):
    drain_inst = self.nc.sync.drain()
    wait_clock.add_sem_waits(
        drain_inst.ins, ScopedClock({None: tick_clock.global_clock})
    )
    si = drain_inst.ins.sync_info
    ow = list(si.on_wait) if si is not None else []
    if len(ow) > 1:
        si.on_wait = [ow[0]]
        for w in ow[1:]:
            d2 = self.nc.sync.drain()
            d2.ins.sync_info = SyncInfo(on_wait=[w], on_update=[])
    self.nc.all_engine_barrier()
    assert self.sems is not None
    popped = self.nc._tile_sem_poison_stack.pop()
    assert popped is self._sem_poison
    self.nc.clear_and_free_semaphores(list(self.sems.allocated().values()))
    self.nc.all_engine_barrier()


tile.TileContext._drain_and_barrier = _drain_and_barrier


def _split_multiwait_instructions(nc):
    """Walrus here allows only one sync-wait per instruction: hoist extra
    waits onto same-engine NOPs inserted immediately before."""
    ctr = 0
    for fn in nc.m.functions:
        for bb in fn.blocks:
            insts = bb.instructions
            out = []
            changed = False
            for ins in insts:
                si = getattr(ins, "sync_info", None)
                ow = list(si.on_wait) if si is not None else []
                if len(ow) > 1:
                    changed = True
                    for w in ow[:-1]:
                        ctr += 1
                        out.append(
                            mybir.InstNoOp(
                                name=f"mwsplit-{ctr}",
                                sync_info=SyncInfo(on_wait=[w], on_update=[]),
                                engine=ins.engine,
                                bass_nofuse=True,
                            )
                        )
                    si.on_wait = [ow[-1]]
                out.append(ins)
            if changed:
                bb.instructions = out
# ---------------------------------------------------------------------------

N_CORES = 8
B, S, IN, OUT, FP = 4, 2048, 4096, 4096, 256
INT = IN - FP                    # 3840 int features (compact order)
NT = (B * S) // N_CORES          # 1024 tokens per core
P = 128
KC = INT // P                    # 30 feature chunks of 128
CC = KC // 2                     # 15 DoubleRow chunks of 256
NOUT = 8                         # out-feature chunks
NSZ = OUT // NOUT                # 512
NGRP = 2                         # n-chunk groups (4 chunks each, 4 psum banks)
GS = NOUT // NGRP                # 4
HT = 2                           # token halves (512) for quantize layout
HSZ = NT // HT                   # 512
TOKT = NT // P                   # 8 token tiles of 128
TH = TOKT // HT                  # 4 token tiles per half

# Measured: the vector engine's f32->i8 output conversion is round-to-
# nearest-even, matching jnp.round exactly; no truncation compensation.
TRUNC_I8 = False

f16 = mybir.dt.float16
f32 = mybir.dt.float32
f8 = mybir.dt.float8e4
i8 = mybir.dt.int8

_prog_cache = {}


def _build_program():
    nc = bass.Bass()
    xst = nc.declare_dram_parameter("xst", [INT, NT], f16, isOutput=False)
    fpxt = nc.declare_dram_parameter("fpxt", [FP, NT], f16, isOutput=False)
    w8n = nc.declare_dram_parameter("w8n", [NOUT, P, CC, 2, NSZ], f8, isOutput=False)
    wfp = nc.declare_dram_parameter("wfp", [FP, OUT], f16, isOutput=False)
    wfp2 = nc.declare_dram_parameter("wfp2", [2, OUT], f16, isOutput=False)
    wsb_d = nc.declare_dram_parameter("wsb", [P, OUT], f16, isOutput=False)
    ident_d = nc.declare_dram_parameter("ident", [P, P], f16, isOutput=False)
    out_d = nc.declare_dram_parameter("out", [NT, OUT], f16, isOutput=True)

    with tile.TileContext(nc) as tc:
        with (
            tc.tile_pool(name="const", bufs=1) as cpool,
            tc.tile_pool(name="xt", bufs=1) as xtpool,
            tc.tile_pool(name="rt", bufs=1) as rtpool,
            tc.tile_pool(name="wp", bufs=1) as wpool,
            tc.tile_pool(name="st", bufs=2) as stpool,
            tc.tile_pool(name="s1", bufs=1) as s1pool,
            tc.tile_pool(name="q", bufs=3) as qpool,
            tc.tile_pool(name="dq", bufs=4) as dqpool,
            tc.tile_pool(name="psA", bufs=2, space="PSUM") as ppoolA,
            tc.tile_pool(name="psB", bufs=1, space="PSUM") as ppoolB,
            tc.tile_pool(name="tr", bufs=1, space="PSUM") as trpool,
            tc.tile_pool(name="dram", bufs=1, space="DRAM") as dpool,
        ):
            # ---- sync ring: all dependency-free loads, priority order ----
            # Loads alternate between the two HWDGE rings (sync/scalar) in
            # need-by order: xt half0 + weight quarter 0 first, remaining
            # weight quarters next, fp/dequant constants, then xt half1.
            rings = [nc.sync, nc.scalar]

            def load_xt(h):
                hs = slice(h * HSZ, (h + 1) * HSZ)
                tiles = []
                for k in range(KC):
                    t_ = xtpool.tile([P, HSZ], f16, name=f"xt{k}", tag=f"xt{k}")
                    rings[k % 2].dma_start(t_[:], xst[k * P : (k + 1) * P, hs])
                    tiles.append(t_)
                return tiles

            QR0 = [(0, 4), (4, 8), (8, 12), (12, CC)]

            def load_wq(g, wq):
                for qi in range(4):
                    c0, c1 = QR0[qi]
                    for s in range(GS):
                        wt = wpool.tile(
                            [P, c1 - c0, 2, NSZ], f8,
                            name=f"w{s}q{qi}", tag=f"w{s}q{qi}",
                        )
                        rings[s % 2].dma_start(wt[:], w8n[g * GS + s, :, c0:c1])
                        wq[s][qi] = wt

            # Early loads: only what the stats + first GEMM chunks need
            # (xt half0, weight quarters 0-1, identity, fp rows). Everything
            # needed later is emitted behind the scalar ring's first
            # dependent compute op, which throttles those transfers until
            # ~the stats are done, keeping early bandwidth on xt.
            ident = cpool.tile([P, P], f16, tag="ident")
            nc.scalar.dma_start(ident[:], ident_d[:])
            xts = [None, None]
            xts[0] = load_xt(0)
            wqs = [[None] * 4 for _ in range(GS)]
            for qi in (0, 1):
                c0, c1 = QR0[qi]
                for s in range(GS):
                    wt = wpool.tile(
                        [P, c1 - c0, 2, NSZ], f8,
                        name=f"w{s}q{qi}", tag=f"w{s}q{qi}",
                    )
                    rings[s % 2].dma_start(wt[:], w8n[s, :, c0:c1])
                    wqs[s][qi] = wt
            fpx0 = cpool.tile([P, NT], f16, tag="fpx0")
            nc.sync.dma_start(fpx0[:], fpxt[0:P, :])
            fpx1 = cpool.tile([P, NT], f16, tag="fpx1")
            nc.scalar.dma_start(fpx1[:], fpxt[P:FP, :])
            wfp0_s = cpool.tile([P, OUT], f16, tag="wfp0")
            wfp1_s = cpool.tile([P, OUT], f16, tag="wfp1")
            wfp2_s = cpool.tile([2, OUT], f16, tag="wfp2")
            wsB = cpool.tile([P, OUT], f16, tag="wsB")

            gate = cpool.tile([1, 1], f32, tag="gate")

            def load_late(rwa):
                # emitted inside half_quant(0). The gate DMA blocks the sync
                # ring's FIFO until the half-0 stats rows exist, so these
                # transfers don't steal early bandwidth from the xt tiles.
                nc.sync.dma_start(gate[:], rwa[0:1, 0:1])
                for qi in (2, 3):
                    c0, c1 = QR0[qi]
                    for s in range(GS):
                        wt = wpool.tile(
                            [P, c1 - c0, 2, NSZ], f8,
                            name=f"w{s}q{qi}", tag=f"w{s}q{qi}",
                        )
                        rings[s % 2].dma_start(wt[:], w8n[s, :, c0:c1])
                        wqs[s][qi] = wt
                nc.sync.dma_start(
                    wfp0_s[:, 0 : OUT // 2], wfp[0:P, 0 : OUT // 2]
                )
                nc.scalar.dma_start(wfp0_s[:, OUT // 2 :], wfp[0:P, OUT // 2 :])
                nc.sync.dma_start(
                    wfp1_s[:, 0 : OUT // 2], wfp[P:FP, 0 : OUT // 2]
                )
                nc.scalar.dma_start(wfp1_s[:, OUT // 2 :], wfp[P:FP, OUT // 2 :])
                nc.sync.dma_start(wfp2_s[:], wfp2[:])
                nc.sync.dma_start(wsB[:, 0 : OUT // 2], wsb_d[:, 0 : OUT // 2])
                nc.scalar.dma_start(wsB[:, OUT // 2 :], wsb_d[:, OUT // 2 :])

            fpt2 = cpool.tile([2, NT], f16, tag="fpt2")
            ones_t = cpool.tile([1, P], f32, tag="ones")
            nc.vector.memset(ones_t[:], 1.0)
            ident32 = cpool.tile([P, P], f32, tag="ident32")
            nc.scalar.copy(ident32[:], ident[:])

            rt = [
                [
                    rtpool.tile(
                        [P, 2, HSZ], f8, name=f"rt{c}_{h}", tag=f"rt{c}_{h}"
                    )
                    for h in range(HT)
                ]
                for c in range(CC)
            ]
            scl = [None] * TOKT
            trees = {}
            rows = {}
            trs = {}

            # ---- per-half stats + quantize, split into emission slots ----
            # slot 0: tree part 1; slot 1: tree part 2; slot 2: transposes +
            # reduces + chains + stat stores; slot 3: rows + broadcasts +
            # fp scaling + quantize. Emitting the slots of half 1 between
            # phase-M(g0) token iterations keeps every engine stream
            # drained while half-0 GEMM work proceeds.
            def half_tree(h, part):
                xt = xts[h]
                if part == 0:
                    mna = stpool.tile([P, HSZ], f16, tag="mna")
                    nc.vector.tensor_tensor(
                        mna[:], xt[0][:], xt[1][:], AluOpType.min
                    )
                    mxa = stpool.tile([P, HSZ], f16, tag="mxa")
                    nc.vector.tensor_tensor(
                        mxa[:], xt[0][:], xt[1][:], AluOpType.max
                    )
                    trees[h] = (mna, mxa)
                    rng = range(2, KC // 2)
                else:
                    mna, mxa = trees[h]
                    rng = range(KC // 2, KC)
                for k in rng:
                    nc.vector.tensor_tensor(mna[:], mna[:], xt[k][:], AluOpType.min)
                    nc.vector.tensor_tensor(mxa[:], mxa[:], xt[k][:], AluOpType.max)
            def half_stats(h):
                mna, mxa = trees[h]
                # psum row tiles for transposed mnq/inv stat columns (f32,
                # partition 0); two pi2 rotations, no extra PSUM bank
                rowtA = ppoolA.tile([P, NSZ], f32, name="pi2", tag="pi2")
                rowtB = ppoolA.tile([P, NSZ], f32, name="pi2", tag="pi2")
                rows[h] = (rowtA, rowtB)
                # PE transpose both accumulators into one psum bank
                tr_t = trpool.tile([P, 2 * HSZ], f16, tag="tr")
                trs[h] = tr_t
                for b in range(TH):
                    bs = slice(b * P, (b + 1) * P)
                    nc.tensor.transpose(tr_t[:, bs], mna[:, bs], ident[:])
                for b in range(TH):
                    bs = slice(b * P, (b + 1) * P)
                    nc.tensor.transpose(
                        tr_t[:, HSZ + b * P : HSZ + (b + 1) * P],
                        mxa[:, bs], ident[:],
                    )
                # per-token-tile stat columns + scale chain
                for b in range(TH):
                    t = h * TH + b
                    mn_t = s1pool.tile([P, 1], f32, name=f"mn{t}", tag=f"mn{t}")
                    nc.vector.tensor_reduce(
                        mn_t[:], tr_t[:, b * P : (b + 1) * P],
                        mybir.AxisListType.X, AluOpType.min,
                    )
                    mx_t = s1pool.tile([P, 1], f32, name=f"mx{t}", tag=f"mx{t}")
                    nc.vector.tensor_reduce(
                        mx_t[:], tr_t[:, HSZ + b * P : HSZ + (b + 1) * P],
                        mybir.AxisListType.X, AluOpType.max,
                    )
                    sc_t = s1pool.tile([P, 1], f32, name=f"sc{t}", tag=f"sc{t}")
                    nc.vector.tensor_tensor(
                        sc_t[:], mx_t[:], mn_t[:], AluOpType.subtract
                    )
                    nc.vector.tensor_scalar(
                        sc_t[:], sc_t[:], 1.0 / 15.0, 1e-8,
                        AluOpType.mult, AluOpType.max,
                    )
                    inv_t = s1pool.tile([P, 1], f32, name=f"inv{t}", tag=f"inv{t}")
                    nc.vector.reciprocal(inv_t[:], sc_t[:])
                    nwt = s1pool.tile([P, 1], f32, name=f"nw{t}", tag=f"nw{t}")
                    nc.vector.tensor_tensor(nwt[:], sc_t[:], inv_t[:], AluOpType.mult)
                    nc.vector.tensor_scalar(
                        nwt[:], nwt[:], -1.0, 2.0, AluOpType.mult, AluOpType.add
                    )
                    nc.vector.tensor_tensor(inv_t[:], inv_t[:], nwt[:], AluOpType.mult)
                    mq_t = s1pool.tile([P, 1], f32, name=f"mq{t}", tag=f"mq{t}")
                    if TRUNC_I8:
                        nc.vector.tensor_scalar(
                            mq_t[:], sc_t[:], -0.5, None, AluOpType.mult
                        )
                        nc.vector.tensor_tensor(
                            mq_t[:], mq_t[:], mn_t[:], AluOpType.add
                        )
                    else:
                        nc.vector.tensor_scalar(
                            mq_t[:], mn_t[:], 1.0, None, AluOpType.mult
                        )
                    zr_t = s1pool.tile([P, 1], f32, name=f"zrf{t}", tag=f"zrf{t}")
                    nc.vector.tensor_tensor(zr_t[:], mn_t[:], inv_t[:], AluOpType.mult)
                    zr16 = s1pool.tile([P, 1], f16, name=f"zr{t}", tag=f"zr{t}")
                    nc.vector.tensor_scalar(
                        zr16[:], zr_t[:], 1.0, 8.0, AluOpType.mult, AluOpType.add
                    )
                    iv16 = s1pool.tile([P, 1], f16, name=f"iv{t}", tag=f"iv{t}")
                    nc.vector.tensor_scalar(
                        iv16[:], inv_t[:], 0.0, None, AluOpType.add
                    )
                    scl[t] = sc_t[:, 0:1]
                    # PE-transpose the stat columns into row form (all at
                    # partition 0)
                    bs = slice(b * P, (b + 1) * P)
                    nc.tensor.transpose(rowtA[0:1, bs], mq_t[:], ident32[:])
                    nc.tensor.transpose(rowtB[0:1, bs], inv_t[:], ident32[:])
                    nc.tensor.transpose(tr_t[0:1, bs], zr16[:], ident[:])
                    nc.tensor.transpose(
                        tr_t[0:1, HSZ + b * P : HSZ + (b + 1) * P], iv16[:],
                        ident[:],
                    )

            def half_quant(h):
                hs = slice(h * HSZ, (h + 1) * HSZ)
                xt = xts[h]
                rowtA, rowtB = rows[h]
                tr_t = trs[h]
                # psum rows -> SBUF (partition-0 aligned copies)
                rwa = stpool.tile([1, HSZ], f32, tag="rwa")
                nc.vector.tensor_scalar(
                    rwa[:], rowtA[0:1, :], 0.0, None, AluOpType.add
                )
                rwb = stpool.tile([1, HSZ], f32, tag="rwb")
                nc.vector.tensor_scalar(
                    rwb[:], rowtB[0:1, :], 0.0, None, AluOpType.add
                )
                if h == 0:
                    load_late(rwa)
                row16 = stpool.tile([1, 2 * HSZ], f16, tag="row16")
                nc.scalar.copy(row16[:], tr_t[0:1, :])
                nc.scalar.dma_start(fpt2[0:1, hs], row16[0:1, 0:HSZ])
                nc.scalar.dma_start(fpt2[1:2, hs], row16[0:1, HSZ:])
                bc0 = ppoolA.tile([P, NSZ], f32, name="pi0", tag="pi0")
                nc.tensor.matmul(
                    bc0[:], ones_t[:], rwa[:], start=True, stop=True
                )
                mnqB = stpool.tile([P, HSZ], f32, tag="mnqB")
                nc.vector.tensor_scalar(mnqB[:], bc0[:], 0.0, None, AluOpType.add)
                bc1 = ppoolA.tile([P, NSZ], f32, name="pi1", tag="pi1")
                nc.tensor.matmul(
                    bc1[:], ones_t[:], rwb[:], start=True, stop=True
                )
                invB = stpool.tile([P, HSZ], f32, tag="invB")
                nc.vector.tensor_scalar(invB[:], bc1[:], 0.0, None, AluOpType.add)
                # fp outlier rows scaled by inv (in place)
                nc.vector.tensor_tensor(
                    fpx0[:, hs], fpx0[:, hs], invB[:], AluOpType.mult
                )
                nc.vector.tensor_tensor(
                    fpx1[:, hs], fpx1[:, hs], invB[:], AluOpType.mult
                )
                # quantize chunks: sub+mult->i8 on vector, i8->f8 on scalar
                for k in range(KC):
                    q = qpool.tile([P, HSZ], f32, name="q", tag="q")
                    nc.vector.tensor_tensor(
                        q[:], xt[k][:], mnqB[:], AluOpType.subtract
                    )
                    r8i = qpool.tile([P, HSZ], i8, name="r8", tag="r8")
                    nc.vector.tensor_tensor(r8i[:], q[:], invB[:], AluOpType.mult)
                    nc.scalar.activation(
                        rt[k // 2][h][:, k % 2, :], r8i[:],
                        mybir.ActivationFunctionType.Copy, bias=-8.0,
                    )

            # ---- phase M: one token-tile iteration ----------------------
            # weights come as 4 quarter-tiles per n-chunk slot (c-ranges
            # 0:4, 4:8, 8:12, 12:15) so group-1 loads can start as soon as
            # group-0's early c-chunks retire.
            QR = [(0, 4), (4, 8), (8, 12), (12, CC)]

            def mm_c(ps, t, c, s, wq, start):
                h = t // TH
                tsl = slice((t % TH) * P, (t % TH) * P + P)
                qi = min(c // 4, 3)
                nc.tensor.matmul(
                    ps[:], rt[c][h][:, :, tsl], wq[s][qi][:, c - QR[qi][0]],
                    start=start, stop=False,
                    perf_mode=mybir.MatmulPerfMode.DoubleRow,
                )

            def mm_fp(ps, g, t, s):
                ts_ = slice(t * P, (t + 1) * P)
                ns = slice((g * GS + s) * NSZ, (g * GS + s + 1) * NSZ)
                nc.tensor.matmul(
                    ps[:], fpx0[:, ts_], wfp0_s[:, ns], start=False, stop=False
                )
                nc.tensor.matmul(
                    ps[:], fpx1[:, ts_], wfp1_s[:, ns], start=False, stop=False
                )
                nc.tensor.matmul(
                    ps[:], fpt2[:, ts_], wfp2_s[:, ns], start=False, stop=True
                )

            def dequant(ps, g, t, s):
                ts_ = slice(t * P, (t + 1) * P)
                ns = slice((g * GS + s) * NSZ, (g * GS + s + 1) * NSZ)
                td = dqpool.tile([P, NSZ], f16, tag="td")
                nc.scalar.activation(
                    td[:], ps[:], mybir.ActivationFunctionType.Copy,
                    scale=scl[t],
                )
                outt = dqpool.tile([P, NSZ], f16, tag="outt")
                nc.vector.tensor_tensor(
                    outt[:], td[:], wsB[:, ns], AluOpType.mult
                )
                nc.scalar.dma_start(out_d[ts_, ns], outt[:])

            def phase_m_t(g, t, wq):
                psums = []
                for s in range(GS):
                    pool = ppoolA if s < 3 else ppoolB
                    psums.append(
                        pool.tile([P, NSZ], f32, name=f"pi{s}", tag=f"pi{s}")
                    )
                for c in range(CC):
                    for s in range(GS):
                        mm_c(psums[s], t, c, s, wq, c == 0)
                for s in range(GS):
                    mm_fp(psums[s], g, t, s)
                for s in range(GS):
                    dequant(psums[s], g, t, s)

            def phase_m_pair(g, ta, tb, wq):
                """Interleave two token-tiles' c-loops (7 matmuls per chunk)
                so a quantize-paced rt stream keeps the tensor engine fed.
                tb's s=3 group runs after ta's s=3 psum bank is drained."""
                psa = []
                for s in range(GS):
                    pool = ppoolA if s < 3 else ppoolB
                    psa.append(
                        pool.tile([P, NSZ], f32, name=f"pi{s}", tag=f"pi{s}")
                    )
                psb = [
                    ppoolA.tile([P, NSZ], f32, name=f"pi{s}", tag=f"pi{s}")
                    for s in range(3)
                ]
                for c in range(CC):
                    for s in range(GS):
                        mm_c(psa[s], ta, c, s, wq, c == 0)
                    for s in range(3):
                        mm_c(psb[s], tb, c, s, wq, c == 0)
                for s in range(GS):
                    mm_fp(psa[s], g, ta, s)
                for s in range(3):
                    mm_fp(psb[s], g, tb, s)
                for s in range(GS):
                    dequant(psa[s], g, ta, s)
                for s in range(3):
                    dequant(psb[s], g, tb, s)
                ps3 = ppoolB.tile([P, NSZ], f32, name="pi3", tag="pi3")
                for c in range(CC):
                    mm_c(ps3, tb, c, 3, wq, c == 0)
                mm_fp(ps3, g, tb, 3)
                dequant(ps3, g, tb, 3)

            # ---- schedule ----------------------------------------------
            half_tree(0, 0)
            half_tree(0, 1)
            half_stats(0)
            half_quant(0)
            xts[1] = load_xt(1)
            phase_m_pair(0, 0, 1, wqs)
            half_tree(1, 0)
            phase_m_t(0, 2, wqs)
            half_tree(1, 1)
            phase_m_t(0, 3, wqs)
            half_stats(1)
            half_quant(1)
            phase_m_pair(0, 4, 5, wqs)
            phase_m_t(0, 6, wqs)
            phase_m_t(0, 7, wqs)
            wqs2 = [[None] * 4 for _ in range(GS)]
            load_wq(1, wqs2)
            for t in range(TOKT):
                phase_m_t(1, t, wqs2)
    _split_multiwait_instructions(nc)
    return nc


def _get_program():
    if "nc" not in _prog_cache:
        _prog_cache["nc"] = _build_program()
    return _prog_cache["nc"]


def _prep_shared(int_weight, fp_weight, bias, weights_scales, reduced_w):
    """Host-side weight layouts (shared across cores)."""
    wint = np.asarray(int_weight).astype(np.float32)          # [OUT, INT]
    ws32 = np.asarray(weights_scales, dtype=np.float32).reshape(OUT, 1)
    # w8n[n, p, c, j, o'] = wint[n*NSZ+o', c*256 + j*128 + p]
    wT = np.ascontiguousarray(wint.T)                         # [INT, OUT]
    w8 = wT.reshape(CC, 2, P, NOUT, NSZ).transpose(3, 2, 0, 1, 4)
    w8n = np.ascontiguousarray(w8).astype(ml_dtypes.float8_e4m3)
    # fp weights scaled by 1/ws, transposed
    wfpT = (np.asarray(fp_weight, dtype=np.float32) / ws32).T  # [FP, OUT]
    wfp = np.ascontiguousarray(wfpT).astype(np.float16)
    # extra contraction rows: [rw/ws = sum_k wint (exact ints), bias/ws]
    row_rw = wint.sum(axis=1)                                  # [OUT]
    row_bias = np.asarray(bias, dtype=np.float32) / ws32[:, 0]
    wfp2 = np.stack([row_rw, row_bias]).astype(np.float16)
    wsb = np.ascontiguousarray(
        np.broadcast_to(
            np.asarray(weights_scales, dtype=np.float16).reshape(1, OUT),
            (P, OUT),
        )
    )
    ident = np.eye(P, dtype=np.float16)
    return w8n, wfp, wfp2, wsb, ident


def _make_in_maps(x, int_weight, fp_weight, bias, weights_scales, reduced_w,
                  int_indices, fp_indices):
    x2 = np.asarray(x, dtype=np.float16).reshape(-1, IN)
    ii = np.asarray(int_indices).astype(np.int64)
    fi = np.asarray(fp_indices).astype(np.int64)

    w8n, wfp, wfp2, wsb, ident = _prep_shared(
        int_weight, fp_weight, bias, weights_scales, reduced_w
    )
    xint = x2[:, ii]                                           # [N, INT]
    xfp = x2[:, fi]                                            # [N, FP]

    in_maps = []
    for c in range(N_CORES):
        sl = slice(c * NT, (c + 1) * NT)
        in_maps.append({
            "xst": np.ascontiguousarray(xint[sl].T),
            "fpxt": np.ascontiguousarray(xfp[sl].T),
            "w8n": w8n,
            "wfp": wfp,
            "wfp2": wfp2,
            "wsb": wsb,
            "ident": ident,
        })
    return in_maps


def kernel(x, int_weight, fp_weight, bias, weights_scales, reduced_w,
           int_indices, fp_indices):
    in_maps = _make_in_maps(
        x, int_weight, fp_weight, bias, weights_scales, reduced_w,
        int_indices, fp_indices,
    )
    nc = _get_program()
    res = run_bass_kernel_spmd(nc, in_maps, list(range(N_CORES)))
    out = np.concatenate(
        [res.results[c]["out"] for c in range(N_CORES)], axis=0
    )
    return out.reshape(B, S, OUT).astype(np.float16)


# revision 39
# speedup vs baseline: 1.0877x; 1.0531x over previous
"""MixedQLinear (QUIK-style int4+fp16 outlier linear) on 8 TRN2 NeuronCores.

Sharding: token-parallel. x [4,2048,4096] -> 8192 tokens, 1024 per core;
weights replicated. Each core quantizes its tokens, runs the int4 GEMM in
fp8e4 DoubleRow mode (exact: products of ints in [-8,7] are exact in the
e6m3 PE datapath, accumulated in fp32 PSUM), and writes its [1024,4096]
output slice. Host concatenates.

Key algebra: with r = clip(round((x-mn)/scale),0,15) - 8,
  out = [ sum_k r*Wint + (fp_x/scale) @ (Wfp/ws)^T + (8+mn/scale)*(rw/ws)
          + (1/scale)*(bias/ws) ] * scale * ws
so the zero-point correction and bias ride as extra contraction rows of the
fp-outlier matmul (scaled by 1/scale per token), and dequant is one scaled
PSUM copy plus one multiply by ws.

Schedule notes (from trace analysis):
- Each DMA instruction drains one queue at ~25 GB/s; the HWDGE rings execute
  their DMA triggers in FIFO order. The sync ring therefore carries ONLY
  dependency-free loads (split into sub-512KB pieces, priority-ordered);
  dependent transfers ride the scalar ring.
- Per-token min/max comes from the transposed x tiles (the same tiles the
  quantizer reads): elementwise min/max trees split across Vector and
  GpSimd, a PE-transpose of the [128,512] accumulators into PSUM, then
  per-token-tile free-axis reduces. Broadcast rows for the quantizer are
  produced by ones[1,128] (x) row[1,512] matmuls into spare PSUM rotations.
- The int GEMM interleaves 4 psum banks per rt-chunk (stationary shared);
  measured matmul issue rate is ~216-222 ns per 512-col stream for both f16
  and DoubleRow. Phase M is emitted as (g0:t0-3), then half-1 stats, then
  (g0:t4-7), (g1:*) so the tensor stream never waits on half-1 stats.
"""

import numpy as np
import ml_dtypes
import concourse.bass as bass
import concourse.tile as tile
import concourse.mybir as mybir
from concourse.bass_utils import run_bass_kernel_spmd
from bass_rust import ScopedClock, SyncInfo
from concourse.alu_op_type import AluOpType

# ---------------------------------------------------------------------------
# Workaround: this toolchain's walrus accepts at most one sync-wait on a
# TPB_CTRL (Drain) instruction; Tile's tail drain attaches one wait per
# active DMA queue. Split it into a chain of single-wait drains.
def _drain_and_barrier(self, tick_clock, wait_clock):
    drain_inst = self.nc.sync.drain()
    wait_clock.add_sem_waits(
        drain_inst.ins, ScopedClock({None: tick_clock.global_clock})
    )
    si = drain_inst.ins.sync_info
    ow = list(si.on_wait) if si is not None else []
    if len(ow) > 1:
        si.on_wait = [ow[0]]
        for w in ow[1:]:
            d2 = self.nc.sync.drain()
            d2.ins.sync_info = SyncInfo(on_wait=[w], on_update=[])
    self.nc.all_engine_barrier()
    assert self.sems is not None
    popped = self.nc._tile_sem_poison_stack.pop()
    assert popped is self._sem_poison
    self.nc.clear_and_free_semaphores(list(self.sems.allocated().values()))
    self.nc.all_engine_barrier()


tile.TileContext._drain_and_barrier = _drain_and_barrier


def _split_multiwait_instructions(nc):
    """Walrus here allows only one sync-wait per instruction: hoist extra
    waits onto same-engine NOPs inserted immediately before."""
    ctr = 0
    for fn in nc.m.functions:
        for bb in fn.blocks:
            insts = bb.instructions
            out = []
            changed = False
            for ins in insts:
                si = getattr(ins, "sync_info", None)
                ow = list(si.on_wait) if si is not None else []
                if len(ow) > 1:
                    changed = True
                    for w in ow[:-1]:
                        ctr += 1
                        out.append(
                            mybir.InstNoOp(
                                name=f"mwsplit-{ctr}",
                                sync_info=SyncInfo(on_wait=[w], on_update=[]),
                                engine=ins.engine,
                                bass_nofuse=True,
                            )
                        )
                    si.on_wait = [ow[-1]]
                out.append(ins)
            if changed:
                bb.instructions = out
# ---------------------------------------------------------------------------

N_CORES = 8
B, S, IN, OUT, FP = 4, 2048, 4096, 4096, 256
INT = IN - FP                    # 3840 int features (compact order)
NT = (B * S) // N_CORES          # 1024 tokens per core
P = 128
KC = INT // P                    # 30 feature chunks of 128
CC = KC // 2                     # 15 DoubleRow chunks of 256
NOUT = 8                         # out-feature chunks
NSZ = OUT // NOUT                # 512
NGRP = 2                         # n-chunk groups (4 chunks each, 4 psum banks)
GS = NOUT // NGRP                # 4
HT = 2                           # token halves (512) for quantize layout
HSZ = NT // HT                   # 512
TOKT = NT // P                   # 8 token tiles of 128
TH = TOKT // HT                  # 4 token tiles per half

# Measured: the vector engine's f32->i8 output conversion is round-to-
# nearest-even, matching jnp.round exactly; no truncation compensation.
TRUNC_I8 = False

f16 = mybir.dt.float16
f32 = mybir.dt.float32
f8 = mybir.dt.float8e4
i8 = mybir.dt.int8

_prog_cache = {}


def _build_program():
    nc = bass.Bass()
    xst = nc.declare_dram_parameter("xst", [INT, NT], f16, isOutput=False)
    fpxt = nc.declare_dram_parameter("fpxt", [FP, NT], f16, isOutput=False)
    w8n = nc.declare_dram_parameter("w8n", [NOUT, P, CC, 2, NSZ], f8, isOutput=False)
    wfp8 = nc.declare_dram_parameter("wfp8", [P, 2, OUT], f8, isOutput=False)
    wfp2 = nc.declare_dram_parameter("wfp2", [2, OUT], f16, isOutput=False)
    wsb_d = nc.declare_dram_parameter("wsb", [P, OUT], f16, isOutput=False)
    ident_d = nc.declare_dram_parameter("ident", [P, P], f16, isOutput=False)
    out_d = nc.declare_dram_parameter("out", [NT, OUT], f16, isOutput=True)

    with tile.TileContext(nc) as tc:
        with (
            tc.tile_pool(name="const", bufs=1) as cpool,
            tc.tile_pool(name="xt", bufs=1) as xtpool,
            tc.tile_pool(name="rt", bufs=1) as rtpool,
            tc.tile_pool(name="wp", bufs=1) as wpool,
            tc.tile_pool(name="st", bufs=1) as stpool,
            tc.tile_pool(name="s1", bufs=1) as s1pool,
            tc.tile_pool(name="q", bufs=2) as qpool,
            tc.tile_pool(name="dq", bufs=4) as dqpool,
            tc.tile_pool(name="psA", bufs=2, space="PSUM") as ppoolA,
            tc.tile_pool(name="psB", bufs=1, space="PSUM") as ppoolB,
            tc.tile_pool(name="tr", bufs=1, space="PSUM") as trpool,
            tc.tile_pool(name="dram", bufs=1, space="DRAM") as dpool,
        ):
            # ---- sync ring: all dependency-free loads, priority order ----
            # Loads alternate between the two HWDGE rings (sync/scalar) in
            # need-by order: xt half0 + weight quarter 0 first, remaining
            # weight quarters next, fp/dequant constants, then xt half1.
            rings = [nc.sync, nc.scalar]

            def load_xt(h):
                hs = slice(h * HSZ, (h + 1) * HSZ)
                tiles = []
                for c in range(CC):
                    t_ = xtpool.tile(
                        [P, 2, HSZ], f16, name=f"xt{c}", tag=f"xt{c}"
                    )
                    for j in range(2):
                        rings[(2 * c + j) % 2].dma_start(
                            t_[:, j, :],
                            xst[(2 * c + j) * P : (2 * c + j + 1) * P, hs],
                        )
                    tiles.append(t_)
                return tiles

            QR0 = [(0, 4), (4, 8), (8, 12), (12, CC)]

            def load_wq(g, wq):
                for qi in range(4):
                    c0, c1 = QR0[qi]
                    for s in range(GS):
                        wt = wpool.tile(
                            [P, c1 - c0, 2, NSZ], f8,
                            name=f"w{s}q{qi}", tag=f"w{s}q{qi}",
                        )
                        rings[s % 2].dma_start(wt[:], w8n[g * GS + s, :, c0:c1])
                        wq[s][qi] = wt

            # Early loads: only what the stats + first GEMM chunks need
            # (xt half0, weight quarters 0-1, identity, fp rows). Everything
            # needed later is emitted behind the scalar ring's first
            # dependent compute op, which throttles those transfers until
            # ~the stats are done, keeping early bandwidth on xt.
            ident = cpool.tile([P, P], f16, tag="ident")
            nc.scalar.dma_start(ident[:], ident_d[:])
            xts = [None, None]
            xts[0] = load_xt(0)
            wqs = [[None] * 4 for _ in range(GS)]
            for qi in (0, 1):
                c0, c1 = QR0[qi]
                for s in range(GS):
                    wt = wpool.tile(
                        [P, c1 - c0, 2, NSZ], f8,
                        name=f"w{s}q{qi}", tag=f"w{s}q{qi}",
                    )
                    rings[s % 2].dma_start(wt[:], w8n[s, :, c0:c1])
                    wqs[s][qi] = wt
            fpx0 = cpool.tile([P, NT], f16, tag="fpx0")
            nc.sync.dma_start(fpx0[:], fpxt[0:P, :])
            fpx1 = cpool.tile([P, NT], f16, tag="fpx1")
            nc.scalar.dma_start(fpx1[:], fpxt[P:FP, :])
            wfp8_s = cpool.tile([P, 2, OUT], f8, tag="wfp8")
            fpt8 = cpool.tile([P, 2, NT], f8, tag="fpt8")
            wfp2_s = cpool.tile([2, OUT], f16, tag="wfp2")
            wsB = cpool.tile([P, OUT], f16, tag="wsB")

            gate = cpool.tile([1, 1], f32, tag="gate")

            def load_late(rwa):
                # emitted inside half_quant(0). The gate DMA blocks the sync
                # ring's FIFO until the half-0 stats rows exist, so these
                # transfers don't steal early bandwidth from the xt tiles.
                nc.sync.dma_start(gate[:], rwa[0:1, 0:1])
                for qi in (2, 3):
                    c0, c1 = QR0[qi]
                    for s in range(GS):
                        wt = wpool.tile(
                            [P, c1 - c0, 2, NSZ], f8,
                            name=f"w{s}q{qi}", tag=f"w{s}q{qi}",
                        )
                        rings[s % 2].dma_start(wt[:], w8n[s, :, c0:c1])
                        wqs[s][qi] = wt
                nc.sync.dma_start(wfp8_s[:, 0, :], wfp8[:, 0, :])
                nc.scalar.dma_start(wfp8_s[:, 1, :], wfp8[:, 1, :])
                nc.sync.dma_start(wfp2_s[:], wfp2[:])
                nc.sync.dma_start(wsB[:, 0 : OUT // 2], wsb_d[:, 0 : OUT // 2])
                nc.scalar.dma_start(wsB[:, OUT // 2 :], wsb_d[:, OUT // 2 :])

            fpt2 = cpool.tile([2, NT], f16, tag="fpt2")
            ones_t = cpool.tile([1, P], f32, tag="ones")
            nc.vector.memset(ones_t[:], 1.0)
            ident32 = cpool.tile([P, P], f32, tag="ident32")
            nc.scalar.copy(ident32[:], ident[:])

            rt = [
                [
                    rtpool.tile(
                        [P, 2, HSZ], f8, name=f"rt{c}_{h}", tag=f"rt{c}_{h}"
                    )
                    for h in range(HT)
                ]
                for c in range(CC)
            ]
            scl = [None] * TOKT
            trees = {}
            rows = {}
            trs = {}

            # ---- per-half stats + quantize, split into emission slots ----
            # slot 0: tree part 1; slot 1: tree part 2; slot 2: transposes +
            # reduces + chains + stat stores; slot 3: rows + broadcasts +
            # fp scaling + quantize. Emitting the slots of half 1 between
            # phase-M(g0) token iterations keeps every engine stream
            # drained while half-0 GEMM work proceeds.
            def half_tree(h, part):
                xt = xts[h]
                if part == 0:
                    mna = stpool.tile([P, 2, HSZ], f16, tag="mna")
                    nc.vector.tensor_tensor(
                        mna[:], xt[0][:], xt[1][:], AluOpType.min
                    )
                    mxa = stpool.tile([P, 2, HSZ], f16, tag="mxa")
                    nc.vector.tensor_tensor(
                        mxa[:], xt[0][:], xt[1][:], AluOpType.max
                    )
                    trees[h] = (mna, mxa)
                    rng = range(2, CC // 2)
                else:
                    mna, mxa = trees[h]
                    rng = range(CC // 2, CC)
                for k in rng:
                    nc.vector.tensor_tensor(mna[:], mna[:], xt[k][:], AluOpType.min)
                    nc.vector.tensor_tensor(mxa[:], mxa[:], xt[k][:], AluOpType.max)
            def half_stats(h):
                mnap, mxap = trees[h]
                mna = stpool.tile([P, HSZ], f16, tag="mnaf")
                nc.vector.tensor_tensor(
                    mna[:], mnap[:, 0, :], mnap[:, 1, :], AluOpType.min
                )
                mxa = stpool.tile([P, HSZ], f16, tag="mxaf")
                nc.vector.tensor_tensor(
                    mxa[:], mxap[:, 0, :], mxap[:, 1, :], AluOpType.max
                )
                # psum row tiles for transposed mnq/inv stat columns (f32,
                # partition 0); two pi2 rotations, no extra PSUM bank
                rowtA = ppoolA.tile([P, NSZ], f32, name="pi2", tag="pi2")
                rowtB = ppoolA.tile([P, NSZ], f32, name="pi2", tag="pi2")
                rows[h] = (rowtA, rowtB)
                # PE transpose both accumulators into one psum bank
                tr_t = trpool.tile([P, 2 * HSZ], f16, tag="tr")
                trs[h] = tr_t
                for b in range(TH):
                    bs = slice(b * P, (b + 1) * P)
                    nc.tensor.transpose(tr_t[:, bs], mna[:, bs], ident[:])
                for b in range(TH):
                    bs = slice(b * P, (b + 1) * P)
                    nc.tensor.transpose(
                        tr_t[:, HSZ + b * P : HSZ + (b + 1) * P],
                        mxa[:, bs], ident[:],
                    )
                # per-token-tile stat columns + scale chain
                for b in range(TH):
                    t = h * TH + b
                    mn_t = s1pool.tile([P, 1], f32, name=f"mn{t}", tag=f"mn{t}")
                    nc.vector.tensor_reduce(
                        mn_t[:], tr_t[:, b * P : (b + 1) * P],
                        mybir.AxisListType.X, AluOpType.min,
                    )
                    mx_t = s1pool.tile([P, 1], f32, name=f"mx{t}", tag=f"mx{t}")
                    nc.vector.tensor_reduce(
                        mx_t[:], tr_t[:, HSZ + b * P : HSZ + (b + 1) * P],
                        mybir.AxisListType.X, AluOpType.max,
                    )
                    sc_t = s1pool.tile([P, 1], f32, name=f"sc{t}", tag=f"sc{t}")
                    nc.vector.tensor_tensor(
                        sc_t[:], mx_t[:], mn_t[:], AluOpType.subtract
                    )
                    nc.vector.tensor_scalar(
                        sc_t[:], sc_t[:], 1.0 / 15.0, 1e-8,
                        AluOpType.mult, AluOpType.max,
                    )
                    inv_t = s1pool.tile([P, 1], f32, name=f"inv{t}", tag=f"inv{t}")
                    nc.vector.reciprocal(inv_t[:], sc_t[:])
                    nwt = s1pool.tile([P, 1], f32, name=f"nw{t}", tag=f"nw{t}")
                    nc.vector.tensor_tensor(nwt[:], sc_t[:], inv_t[:], AluOpType.mult)
                    nc.vector.tensor_scalar(
                        nwt[:], nwt[:], -1.0, 2.0, AluOpType.mult, AluOpType.add
                    )
                    nc.vector.tensor_tensor(inv_t[:], inv_t[:], nwt[:], AluOpType.mult)
                    mq_t = s1pool.tile([P, 1], f32, name=f"mq{t}", tag=f"mq{t}")
                    if TRUNC_I8:
                        nc.vector.tensor_scalar(
                            mq_t[:], sc_t[:], -0.5, None, AluOpType.mult
                        )
                        nc.vector.tensor_tensor(
                            mq_t[:], mq_t[:], mn_t[:], AluOpType.add
                        )
                    else:
                        nc.vector.tensor_scalar(
                            mq_t[:], mn_t[:], 1.0, None, AluOpType.mult
                        )
                    zr_t = s1pool.tile([P, 1], f32, name=f"zrf{t}", tag=f"zrf{t}")
                    nc.vector.tensor_tensor(zr_t[:], mn_t[:], inv_t[:], AluOpType.mult)
                    zr16 = s1pool.tile([P, 1], f16, name=f"zr{t}", tag=f"zr{t}")
                    nc.vector.tensor_scalar(
                        zr16[:], zr_t[:], 1.0, 8.0, AluOpType.mult, AluOpType.add
                    )
                    iv16 = s1pool.tile([P, 1], f16, name=f"iv{t}", tag=f"iv{t}")
                    nc.vector.tensor_scalar(
                        iv16[:], inv_t[:], 0.0, None, AluOpType.add
                    )
                    scl[t] = sc_t[:, 0:1]
                    # PE-transpose the stat columns into row form (all at
                    # partition 0)
                    bs = slice(b * P, (b + 1) * P)
                    nc.tensor.transpose(rowtA[0:1, bs], mq_t[:], ident32[:])
                    nc.tensor.transpose(rowtB[0:1, bs], inv_t[:], ident32[:])
                    nc.tensor.transpose(tr_t[0:1, bs], zr16[:], ident[:])
                    nc.tensor.transpose(
                        tr_t[0:1, HSZ + b * P : HSZ + (b + 1) * P], iv16[:],
                        ident[:],
                    )

            def half_quant(h):
                hs = slice(h * HSZ, (h + 1) * HSZ)
                xt = xts[h]
                rowtA, rowtB = rows[h]
                tr_t = trs[h]
                # psum rows -> SBUF (partition-0 aligned copies)
                rwa = stpool.tile([1, HSZ], f32, tag="rwa")
                nc.vector.tensor_scalar(
                    rwa[:], rowtA[0:1, :], 0.0, None, AluOpType.add
                )
                rwb = stpool.tile([1, HSZ], f32, tag="rwb")
                nc.vector.tensor_scalar(
                    rwb[:], rowtB[0:1, :], 0.0, None, AluOpType.add
                )
                if h == 0:
                    load_late(rwa)
                row16 = stpool.tile([1, 2 * HSZ], f16, tag="row16")
                nc.scalar.copy(row16[:], tr_t[0:1, :])
                nc.scalar.dma_start(fpt2[0:1, hs], row16[0:1, 0:HSZ])
                nc.scalar.dma_start(fpt2[1:2, hs], row16[0:1, HSZ:])
                bc0 = ppoolA.tile([P, NSZ], f32, name="pi0", tag="pi0")
                nc.tensor.matmul(
                    bc0[:], ones_t[:], rwa[:], start=True, stop=True
                )
                mnqB = stpool.tile([P, 2, HSZ], f32, tag="mnqB")
                nc.vector.tensor_scalar(
                    mnqB[:, 0, :], bc0[:], 0.0, None, AluOpType.add
                )
                nc.vector.tensor_scalar(
                    mnqB[:, 1, :], bc0[:], 0.0, None, AluOpType.add
                )
                bc1 = ppoolA.tile([P, NSZ], f32, name="pi1", tag="pi1")
                nc.tensor.matmul(
                    bc1[:], ones_t[:], rwb[:], start=True, stop=True
                )
                invB = stpool.tile([P, 2, HSZ], f32, tag="invB")
                nc.vector.tensor_scalar(
                    invB[:, 0, :], bc1[:], 0.0, None, AluOpType.add
                )
                nc.vector.tensor_scalar(
                    invB[:, 1, :], bc1[:], 0.0, None, AluOpType.add
                )
                # fp outlier rows scaled by inv, cast to fp8 pairs
                nc.vector.tensor_tensor(
                    fpt8[:, 0, hs], fpx0[:, hs], invB[:, 0, :], AluOpType.mult
                )
                nc.vector.tensor_tensor(
                    fpt8[:, 1, hs], fpx1[:, hs], invB[:, 0, :], AluOpType.mult
                )
                # quantize chunk pairs: sub+mult->i8 on vector, i8->f8 on
                # scalar, one [128,2,512] op per stage
                for c in range(CC):
                    q = qpool.tile([P, 2, HSZ], f32, name="q", tag="q")
                    nc.vector.tensor_tensor(
                        q[:], xt[c][:], mnqB[:], AluOpType.subtract
                    )
                    r8i = qpool.tile([P, 2, HSZ], i8, name="r8", tag="r8")
                    nc.vector.tensor_tensor(r8i[:], q[:], invB[:], AluOpType.mult)
                    nc.scalar.activation(
                        rt[c][h][:], r8i[:],
                        mybir.ActivationFunctionType.Copy, bias=-8.0,
                    )

            # ---- phase M: one token-tile iteration ----------------------
            # weights come as 4 quarter-tiles per n-chunk slot (c-ranges
            # 0:4, 4:8, 8:12, 12:15) so group-1 loads can start as soon as
            # group-0's early c-chunks retire.
            QR = [(0, 4), (4, 8), (8, 12), (12, CC)]

            def mm_c(ps, t, c, s, wq, start):
                h = t // TH
                tsl = slice((t % TH) * P, (t % TH) * P + P)
                qi = min(c // 4, 3)
                nc.tensor.matmul(
                    ps[:], rt[c][h][:, :, tsl], wq[s][qi][:, c - QR[qi][0]],
                    start=start, stop=False,
                    perf_mode=mybir.MatmulPerfMode.DoubleRow,
                )

            def mm_fp(ps, g, t, s):
                ts_ = slice(t * P, (t + 1) * P)
                ns = slice((g * GS + s) * NSZ, (g * GS + s + 1) * NSZ)
                nc.tensor.matmul(
                    ps[:], fpt8[:, :, ts_], wfp8_s[:, :, ns],
                    start=False, stop=False,
                    perf_mode=mybir.MatmulPerfMode.DoubleRow,
                )
                nc.tensor.matmul(
                    ps[:], fpt2[:, ts_], wfp2_s[:, ns], start=False, stop=True
                )

            def dequant(ps, g, t, s):
                ts_ = slice(t * P, (t + 1) * P)
                ns = slice((g * GS + s) * NSZ, (g * GS + s + 1) * NSZ)
                td = dqpool.tile([P, NSZ], f16, tag="td")
                nc.scalar.activation(
                    td[:], ps[:], mybir.ActivationFunctionType.Copy,
                    scale=scl[t],
                )
                outt = dqpool.tile([P, NSZ], f16, tag="outt")
                nc.vector.tensor_tensor(
                    outt[:], td[:], wsB[:, ns], AluOpType.mult
                )
                nc.scalar.dma_start(out_d[ts_, ns], outt[:])

            def phase_m_t(g, t, wq):
                psums = []
                for s in range(GS):
                    pool = ppoolA if s < 3 else ppoolB
                    psums.append(
                        pool.tile([P, NSZ], f32, name=f"pi{s}", tag=f"pi{s}")
                    )
                for c in range(CC):
                    for s in range(GS):
                        mm_c(psums[s], t, c, s, wq, c == 0)
                for s in range(GS):
                    mm_fp(psums[s], g, t, s)
                for s in range(GS):
                    dequant(psums[s], g, t, s)

            def phase_m_pair(g, ta, tb, wq):
                """Interleave two token-tiles' c-loops (7 matmuls per chunk)
                so a quantize-paced rt stream keeps the tensor engine fed.
                tb's s=3 group runs after ta's s=3 psum bank is drained."""
                psa = []
                for s in range(GS):
                    pool = ppoolA if s < 3 else ppoolB
                    psa.append(
                        pool.tile([P, NSZ], f32, name=f"pi{s}", tag=f"pi{s}")
                    )
                psb = [
                    ppoolA.tile([P, NSZ], f32, name=f"pi{s}", tag=f"pi{s}")
                    for s in range(3)
                ]
                for c in range(CC):
                    for s in range(GS):
                        mm_c(psa[s], ta, c, s, wq, c == 0)
                    for s in range(3):
                        mm_c(psb[s], tb, c, s, wq, c == 0)
                for s in range(GS):
                    mm_fp(psa[s], g, ta, s)
                for s in range(3):
                    mm_fp(psb[s], g, tb, s)
                for s in range(GS):
                    dequant(psa[s], g, ta, s)
                for s in range(3):
                    dequant(psb[s], g, tb, s)
                ps3 = ppoolB.tile([P, NSZ], f32, name="pi3", tag="pi3")
                for c in range(CC):
                    mm_c(ps3, tb, c, 3, wq, c == 0)
                mm_fp(ps3, g, tb, 3)
                dequant(ps3, g, tb, 3)

            # ---- schedule ----------------------------------------------
            half_tree(0, 0)
            half_tree(0, 1)
            half_stats(0)
            half_quant(0)
            xts[1] = load_xt(1)
            phase_m_pair(0, 0, 1, wqs)
            half_tree(1, 0)
            phase_m_t(0, 2, wqs)
            half_tree(1, 1)
            phase_m_t(0, 3, wqs)
            half_stats(1)
            half_quant(1)
            phase_m_pair(0, 4, 5, wqs)
            phase_m_t(0, 6, wqs)
            phase_m_t(0, 7, wqs)
            wqs2 = [[None] * 4 for _ in range(GS)]
            load_wq(1, wqs2)
            for t in range(TOKT):
                phase_m_t(1, t, wqs2)
    _split_multiwait_instructions(nc)
    return nc


def _get_program():
    if "nc" not in _prog_cache:
        _prog_cache["nc"] = _build_program()
    return _prog_cache["nc"]


def _prep_shared(int_weight, fp_weight, bias, weights_scales, reduced_w):
    """Host-side weight layouts (shared across cores)."""
    wint = np.asarray(int_weight).astype(np.float32)          # [OUT, INT]
    ws32 = np.asarray(weights_scales, dtype=np.float32).reshape(OUT, 1)
    # w8n[n, p, c, j, o'] = wint[n*NSZ+o', c*256 + j*128 + p]
    wT = np.ascontiguousarray(wint.T)                         # [INT, OUT]
    w8 = wT.reshape(CC, 2, P, NOUT, NSZ).transpose(3, 2, 0, 1, 4)
    w8n = np.ascontiguousarray(w8).astype(ml_dtypes.float8_e4m3)
    # fp weights scaled by 1/ws, transposed, fp8 DoubleRow pair layout
    wfpT = (np.asarray(fp_weight, dtype=np.float32) / ws32).T  # [FP, OUT]
    wfp8 = np.ascontiguousarray(
        np.clip(wfpT, -240.0, 240.0).reshape(2, P, OUT).transpose(1, 0, 2)
    ).astype(ml_dtypes.float8_e4m3)
    # extra contraction rows: [rw/ws = sum_k wint (exact ints), bias/ws]
    row_rw = wint.sum(axis=1)                                  # [OUT]
    row_bias = np.asarray(bias, dtype=np.float32) / ws32[:, 0]
    wfp2 = np.stack([row_rw, row_bias]).astype(np.float16)
    wsb = np.ascontiguousarray(
        np.broadcast_to(
            np.asarray(weights_scales, dtype=np.float16).reshape(1, OUT),
            (P, OUT),
        )
    )
    ident = np.eye(P, dtype=np.float16)
    return w8n, wfp8, wfp2, wsb, ident


def _make_in_maps(x, int_weight, fp_weight, bias, weights_scales, reduced_w,
                  int_indices, fp_indices):
    x2 = np.asarray(x, dtype=np.float16).reshape(-1, IN)
    ii = np.asarray(int_indices).astype(np.int64)
    fi = np.asarray(fp_indices).astype(np.int64)

    w8n, wfp8, wfp2, wsb, ident = _prep_shared(
        int_weight, fp_weight, bias, weights_scales, reduced_w
    )
    xint = x2[:, ii]                                           # [N, INT]
    xfp = x2[:, fi]                                            # [N, FP]

    in_maps = []
    for c in range(N_CORES):
        sl = slice(c * NT, (c + 1) * NT)
        in_maps.append({
            "xst": np.ascontiguousarray(xint[sl].T),
            "fpxt": np.ascontiguousarray(xfp[sl].T),
            "w8n": w8n,
            "wfp8": wfp8,
            "wfp2": wfp2,
            "wsb": wsb,
            "ident": ident,
        })
    return in_maps


def kernel(x, int_weight, fp_weight, bias, weights_scales, reduced_w,
           int_indices, fp_indices):
    in_maps = _make_in_maps(
        x, int_weight, fp_weight, bias, weights_scales, reduced_w,
        int_indices, fp_indices,
    )
    nc = _get_program()
    res = run_bass_kernel_spmd(nc, in_maps, list(range(N_CORES)))
    out = np.concatenate(
        [res.results[c]["out"] for c in range(N_CORES)], axis=0
    )
    return out.reshape(B, S, OUT).astype(np.float16)
